# revision 46
# baseline (speedup 1.0000x reference)
"""Trainium2 Bass kernel for the ANR aspect-attention module (nn_ANR_ARL).

reference math, for review (B=256, L=500, E=300), aspProj (K=5, E, H=10),
aspEmbed (K, 3H):
    proj   = einsum('ble,keh->kblh', review, aspProj)
    scores = window-3 conv of proj against aspEmbed (zero-padded) per (k,b,l)
    attn   = softmax(scores, axis=l)
    rep    = einsum('kblh,kbl->kbh', proj, attn)
returns (attn -> (B,K,L), rep -> (B,K,H)).

Strategy: pure data parallel across 8 NeuronCores (32 batch items each, no
collectives needed in the forward pass). Host prep re-lays review as
(B, E, L) bf16 so the E-contraction maps onto TensorEngine partitions and
HBM traffic is halved, and repacks the tiny parameters into matmul-ready
stationary operands (replicated to all cores). PSUM accumulation is f32.

Batch pairs are PARTITION-packed: row r = 50*bh + (10k+h) for bh in {0,1}.
Matmuls can't write at a PSUM partition offset, so each batch's stationary
operand is zero-padded to the full 100 output rows and the pair is merged
by PSUM accumulation (zero columns are free: matmul cost is N cycles
regardless of M). The two batches of a pair are also stacked into one
600-row (bh, e) contraction space tiled as 5 chunks of 120 rows. Pairs are
processed two at a time (the first group as two single pairs so compute
starts as early as possible) and most vector ops cover 4 batches per pass:
  1. projT[r, l]: 5 accumulating matmuls per pair.
  2. scores[5*bh+k, l]: 3 column-shifted accumulating matmuls per pair
     (window conv folded into PSUM accumulation; edges get exact zero-pad
     semantics).
  3. softmax: ACT exp (accum_out = denominator for free) -> DVE recip ->
     GpSimd scale by 1/Z.
  4. rep: a 0/1 broadcast matmul expands e rows (bh,k) -> (bh,k,h); DVE
     mul + reduce accumulates unnormalized rep; one tiny matmul + DVE mul
     at the end applies all 1/Z factors at once.
"""

import sys

if "/opt/trn_rl_repo" not in sys.path:
    sys.path.insert(0, "/opt/trn_rl_repo")

import numpy as np
import ml_dtypes

B, L, E, H, K = 256, 500, 300, 10, 5
KH = K * H                 # 50 rows per batch
R2 = 2 * KH                # 100 rows per packed pair
N_CORES = 8
BL = B // N_CORES          # 32 batches per core
NPAIR = BL // 2            # 16 batch pairs per core
NQUAD = NPAIR // 2         # 8 quads (4 batches each)
EC = 100                   # E-chunk (contraction) size
PAIRS_PER_CHUNK = 2        # attn output staging granularity

_CACHED = None


def _build():
    import concourse.bacc as bacc
    import concourse.tile as tile
    from concourse import mybir

    f32 = mybir.dt.float32
    bf16 = mybir.dt.bfloat16
    nc = bacc.Bacc("TRN2", target_bir_lowering=False, debug=False,
                   num_devices=N_CORES)

    # pre-tiled on host: [group, partition, pair, chunk, l] so each
    # partition row of a group load is one contiguous run (fast HWDGE
    # descriptor generation)
    reviewt = nc.dram_tensor("reviewt", (NQUAD, 120, 2, 5, L), bf16,
                             kind="ExternalInput")
    wproj_d = nc.dram_tensor("wproj", (120, 5, R2), bf16, kind="ExternalInput")
    wsc_d = nc.dram_tensor("wsc", (R2, 3, 2 * K), bf16, kind="ExternalInput")
    b01_d = nc.dram_tensor("b01", (2 * K, R2), bf16, kind="ExternalInput")
    attn_d = nc.dram_tensor("attn", (BL, K, L), f32, kind="ExternalOutput")
    rep_d = nc.dram_tensor("rep", (BL, K, H), f32, kind="ExternalOutput")

    with tile.TileContext(nc, pool_alloc_mode="queue") as tc:
        with (
            tc.tile_pool(name="consts", bufs=1) as consts,
            tc.tile_pool(name="rv", bufs=9) as rvp,
            tc.tile_pool(name="projT", bufs=3) as projp,
            tc.tile_pool(name="small", bufs=4) as smallp,
            tc.tile_pool(name="prod", bufs=3) as prodp,
            tc.tile_pool(name="attnout", bufs=4) as attnp,
            tc.tile_pool(name="accs", bufs=1) as accs,
            tc.tile_pool(name="ps_big", bufs=4, space="PSUM") as ps_big,
            tc.tile_pool(name="ps_sc", bufs=2, space="PSUM") as ps_sc,
            tc.tile_pool(name="ps_ebc", bufs=1, space="PSUM") as ps_ebc,
            tc.tile_pool(name="ps_warm", bufs=1, space="PSUM") as ps_warm,
        ):

            rep_raw = accs.tile([R2, NPAIR], f32)
            rz_allf = accs.tile([2 * K, NPAIR], f32)

            # attn out view: row r=(bh,k) on partitions -> [r, pair, l]
            attn_view = attn_d[:].rearrange("(pr two) k l -> (two k) pr l", two=2)

            # --- constants ---
            wproj_sb = consts.tile([120, 5, R2], bf16)
            nc.sync.dma_start(out=wproj_sb, in_=wproj_d[:])
            wsc_sb = consts.tile([R2, 3, 2 * K], bf16)
            nc.sync.dma_start(out=wsc_sb, in_=wsc_d[:])
            b01_sb = consts.tile([2 * K, R2], bf16)
            nc.sync.dma_start(out=b01_sb, in_=b01_d[:])
            b01f_d_view = b01_d  # reuse bf16 dram, cast during SWDGE load
            b01f = consts.tile([2 * K, R2], f32)
            nc.gpsimd.dma_start(out=b01f, in_=b01f_d_view[:])

            blocks = []  # (pair0, W, rv tile)
            rv0a = rvp.tile([120, 1, 5, L], bf16, tag="rv")
            nc.sync.dma_start(out=rv0a, in_=reviewt[0, :, 0:1])
            blocks.append((0, 1, rv0a))
            rv0b = rvp.tile([120, 1, 5, L], bf16, tag="rv")
            nc.sync.dma_start(out=rv0b, in_=reviewt[0, :, 1:2])
            blocks.append((1, 1, rv0b))
            for g in range(1, NQUAD):
                rv = rvp.tile([120, 2, 5, L], bf16, tag="rv")
                nc.sync.dma_start(out=rv, in_=reviewt[g])
                blocks.append((2 * g, 2, rv))

            attn_chunk = None
            for pair0, W, rv in blocks:
                pc = pair0 % PAIRS_PER_CHUNK
                if pc == 0:
                    attn_chunk = attnp.tile(
                        [2 * K, PAIRS_PER_CHUNK, L], f32, tag="attn"
                    )

                # 1) projection: combined (bh, e) contraction rows tiled
                #    as 5 chunks of 120; weights loaded once per chunk;
                #    per-pair PSUM tiles for finer pipelining
                pproj_t = [ps_big.tile([R2, 512], f32, tag="pproj",
                                       name=f"pproj{p2}")
                           for p2 in range(W)]
                for c in range(5):
                    for p2 in range(W):
                        nc.tensor.matmul(
                            pproj_t[p2][:, 0:L],
                            wproj_sb[:, c, :],
                            rv[:, p2, c, :],
                            start=(c == 0),
                            stop=(c == 4),
                        )
                projT = projp.tile([R2, W, L], bf16, tag="projT")
                for p2 in range(W):
                    nc.scalar.copy(out=projT[:, p2, :], in_=pproj_t[p2][:, 0:L])

                # 2) scores: window conv as 3 shifted accumulating matmuls
                psc_t = [ps_sc.tile([2 * K, 512], f32, tag="psc",
                                    name=f"psc{p2}")
                         for p2 in range(W)]
                for p2 in range(W):
                    nc.tensor.matmul(
                        psc_t[p2][:, 0:L], wsc_sb[:, 1, :], projT[:, p2, 0:L],
                        start=True, stop=False)
                for p2 in range(W):
                    nc.tensor.matmul(
                        psc_t[p2][:, 1:L], wsc_sb[:, 0, :], projT[:, p2, 0:L - 1],
                        start=False, stop=False, skip_group_check=True)
                for p2 in range(W):
                    nc.tensor.matmul(
                        psc_t[p2][:, 0:L - 1], wsc_sb[:, 2, :], projT[:, p2, 1:L],
                        start=False, stop=True)

                # 3) softmax along l (scores are O(1e-2): no max needed);
                #    exp's accum_out is the denominator
                e_sb = smallp.tile([2 * K, W, L], bf16, tag="e")
                for p2 in range(W):
                    nc.scalar.activation(
                        out=e_sb[:, p2, :], in_=psc_t[p2][:, 0:L],
                        func=mybir.ActivationFunctionType.Exp,
                        accum_out=rz_allf[:, pair0 + p2:pair0 + p2 + 1],
                    )
                rzq = rz_allf[:, pair0:pair0 + W]
                nc.vector.reciprocal(out=rzq, in_=rzq)
                scale_eng = (nc.gpsimd if (pair0 // 2) % 2 == 0 or pair0 + W == NPAIR
                             else nc.vector)
                scale_eng.tensor_tensor(
                    out=attn_chunk[:, pc:pc + W, :], in0=e_sb,
                    in1=rzq.to_broadcast([2 * K, W, L]),
                    op=mybir.AluOpType.mult,
                )

                # 4) rep: broadcast e rows (bh,k) -> (bh,k,h), mul, reduce
                prod = prodp.tile([R2, W, L], f32, tag="prod")
                for p2 in range(W):
                    pebc = ps_ebc.tile([R2, 512], f32, tag="pebc")
                    nc.tensor.matmul(pebc[:, 0:L], b01_sb, e_sb[:, p2, :],
                                     start=True, stop=True)
                    nc.vector.tensor_tensor(
                        out=prod[:, p2, :], in0=projT[:, p2, :],
                        in1=pebc[:, 0:L], op=mybir.AluOpType.mult,
                    )
                nc.vector.tensor_reduce(
                    out=rep_raw[:, pair0:pair0 + W], in_=prod,
                    axis=mybir.AxisListType.X, op=mybir.AluOpType.add,
                )

                if pc + W == PAIRS_PER_CHUNK:
                    qq = pair0 // PAIRS_PER_CHUNK
                    nc.sync.dma_start(
                        out=attn_view[:, qq * PAIRS_PER_CHUNK:(qq + 1) * PAIRS_PER_CHUNK],
                        in_=attn_chunk,
                    )

                # rep normalization rz10[r, pair] = 1/Z[bh(r), k(r), pair]:
                # pairs 0-13 overlapped with compute, last two in the tail
                # (f32 matmul: no bf16 cast hop; N is tiny so rate is moot)
                for h0, h1 in (((0, NPAIR - 2),) if pair0 + W == NPAIR - 2
                               else ((NPAIR - 2, NPAIR),) if pair0 + W == NPAIR
                               else ()):
                    prz = ps_warm.tile([R2, h1 - h0], f32, tag="warm")
                    nc.tensor.matmul(prz, b01f, rz_allf[:, h0:h1],
                                     start=True, stop=True)
                    rep_out = smallp.tile([R2, h1 - h0], f32, tag="rpo")
                    nc.vector.tensor_tensor(out=rep_out, in0=rep_raw[:, h0:h1],
                                            in1=prz, op=mybir.AluOpType.mult)
                    rep_view = rep_d[:].rearrange(
                        "(pr two) k h -> (two k h) pr", two=2)
                    nc.sync.dma_start(out=rep_view[:, h0:h1], in_=rep_out)


            # (rep normalization emitted inside the block loop)

    nc.compile()
    return nc


def _host_prep(review, aspProj, aspEmbed):
    """Build per-core input maps (host-side shard + layout prep)."""
    review = np.asarray(review, dtype=np.float32)
    aspProj = np.asarray(aspProj, dtype=np.float32)
    aspEmbed = np.asarray(aspEmbed, dtype=np.float32)
    bf = ml_dtypes.bfloat16

    # (B, E, L) layout so E lands on SBUF partitions; bf16 halves HBM
    # bytes. The two batches of each pair are stacked into one 600-row
    # (bh, e) contraction space, tiled as 5 chunks of 120, then pre-tiled
    # per core as [group, p, pair, chunk, l] so each partition row of a
    # group load is one contiguous run (fast HWDGE descriptor gen).
    reviewt = review.transpose(0, 2, 1).astype(bf)       # (B, E, L)
    reviewt = reviewt.reshape(B // 2, 2 * E, L)          # pair-stacked rows
    reviewt = reviewt.reshape(B // 2, 5, 120, L)         # 5 chunks of 120
    reviewt = np.ascontiguousarray(
        reviewt.reshape(B // 4, 2, 5, 120, L).transpose(0, 3, 1, 2, 4))

    wflat = aspProj.transpose(1, 0, 2).reshape(E, KH)  # [e, 10k+h]
    # wproj[p, c, 50bh + f] = wflat[e, f] for row g = 120c + p = 300bh + e
    wproj = np.zeros((120, 5, R2), np.float32)
    for c in range(5):
        for p in range(120):
            g = 120 * c + p
            bh, e = divmod(g, E)
            wproj[p, c, KH * bh:KH * (bh + 1)] = wflat[e]

    # wsc[r=50bh+10k+h, o, 5bh+k] = aspEmbed[k, 3h+o]
    wsc = np.zeros((R2, 3, 2 * K), np.float32)
    for bh in range(2):
        for k in range(K):
            for h in range(H):
                wsc[KH * bh + 10 * k + h, :, K * bh + k] = aspEmbed[k, 3 * h:3 * h + 3]

    # b01[5bh+k, 50bh+10k'+h] = (k == k')
    b01 = np.zeros((2 * K, R2), np.float32)
    for bh in range(2):
        for k in range(K):
            b01[K * bh + k, KH * bh + 10 * k:KH * bh + 10 * k + H] = 1.0

    wproj, wsc, b01 = (x.astype(bf) for x in (wproj, wsc, b01))

    in_maps = []
    for core in range(N_CORES):
        in_maps.append({
            "reviewt": np.ascontiguousarray(
                reviewt[core * NQUAD:(core + 1) * NQUAD]),
            "wproj": wproj,
            "wsc": wsc,
            "b01": b01,
        })
    return in_maps


def kernel(review, aspProj, aspEmbed, _trace=False):
    global _CACHED
    from concourse.bass_utils import run_bass_kernel_spmd

    if _CACHED is None:
        _CACHED = _build()
    nc = _CACHED

    in_maps = _host_prep(review, aspProj, aspEmbed)
    res = None
    for attempt in range(3):
        try:
            res = run_bass_kernel_spmd(
                nc, in_maps, core_ids=list(range(N_CORES)), trace=_trace
            )
            break
        except Exception:
            if attempt == 2:
                raise
            import time
            time.sleep(30)

    attn = np.concatenate([res.results[c]["attn"] for c in range(N_CORES)], axis=0)
    rep = np.concatenate([res.results[c]["rep"] for c in range(N_CORES)], axis=0)
    if _trace:
        kernel.last_exec_time_ns = res.exec_time_ns
        kernel.last_results = res
    return attn, rep


# revision 48
# speedup vs baseline: 1.1938x; 1.1938x over previous
"""Trainium2 Bass kernel for the ANR aspect-attention module (nn_ANR_ARL).

reference math, for review (B=256, L=500, E=300), aspProj (K=5, E, H=10),
aspEmbed (K, 3H):
    proj   = einsum('ble,keh->kblh', review, aspProj)
    scores = window-3 conv of proj against aspEmbed (zero-padded) per (k,b,l)
    attn   = softmax(scores, axis=l)
    rep    = einsum('kblh,kbl->kbh', proj, attn)
returns (attn -> (B,K,L), rep -> (B,K,H)).

Strategy: pure data parallel across 8 NeuronCores (32 batch items each, no
collectives needed in the forward pass). Host prep re-lays review as
(B, E, L) bf16 so the E-contraction maps onto TensorEngine partitions and
HBM traffic is halved, and repacks the tiny parameters into matmul-ready
stationary operands (replicated to all cores). PSUM accumulation is f32.

Batch pairs are PARTITION-packed: row r = 50*bh + (10k+h) for bh in {0,1}.
Matmuls can't write at a PSUM partition offset, so each batch's stationary
operand is zero-padded to the full 100 output rows and the pair is merged
by PSUM accumulation (zero columns are free: matmul cost is N cycles
regardless of M). The two batches of a pair are also stacked into one
600-row (bh, e) contraction space tiled as 5 chunks of 120 rows. Pairs are
processed two at a time (the first group as two single pairs so compute
starts as early as possible) and most vector ops cover 4 batches per pass:
  1. projT[r, l]: 5 accumulating matmuls per pair.
  2. scores[5*bh+k, l]: 3 column-shifted accumulating matmuls per pair
     (window conv folded into PSUM accumulation; edges get exact zero-pad
     semantics).
  3. softmax: ACT exp (accum_out = denominator for free) -> DVE recip ->
     GpSimd scale by 1/Z.
  4. rep: a 0/1 broadcast matmul expands e rows (bh,k) -> (bh,k,h); DVE
     mul + reduce accumulates unnormalized rep; one tiny matmul + DVE mul
     at the end applies all 1/Z factors at once.
"""

import sys

if "/opt/trn_rl_repo" not in sys.path:
    sys.path.insert(0, "/opt/trn_rl_repo")

import numpy as np
import ml_dtypes

B, L, E, H, K = 256, 500, 300, 10, 5
KH = K * H                 # 50 rows per batch
R2 = 2 * KH                # 100 rows per packed pair
N_CORES = 8
BL = B // N_CORES          # 32 batches per core
NPAIR = BL // 2            # 16 batch pairs per core
NQUAD = NPAIR // 2         # 8 quads (4 batches each)
EC = 100                   # E-chunk (contraction) size
PAIRS_PER_CHUNK = 2        # attn output staging granularity

_CACHED = None


def _build():
    import concourse.bacc as bacc
    import concourse.tile as tile
    from concourse import mybir

    f32 = mybir.dt.float32
    bf16 = mybir.dt.bfloat16
    nc = bacc.Bacc("TRN2", target_bir_lowering=False, debug=False,
                   num_devices=N_CORES)

    # pre-tiled on host: [group, partition, pair, chunk, l] so each
    # partition row of a group load is one contiguous run (fast HWDGE
    # descriptor generation)
    reviewt = nc.dram_tensor("reviewt", (NQUAD, 120, 2, 5, L), bf16,
                             kind="ExternalInput")
    wproj_d = nc.dram_tensor("wproj", (120, 5, R2), bf16, kind="ExternalInput")
    wsc_d = nc.dram_tensor("wsc", (R2, 3, 2 * K), bf16, kind="ExternalInput")
    b01_d = nc.dram_tensor("b01", (2 * K, R2), bf16, kind="ExternalInput")
    attn_d = nc.dram_tensor("attn", (BL, K, L), f32, kind="ExternalOutput")
    rep_d = nc.dram_tensor("rep", (BL, K, H), f32, kind="ExternalOutput")

    with tile.TileContext(nc, pool_alloc_mode="queue") as tc:
        with (
            tc.tile_pool(name="consts", bufs=1) as consts,
            tc.tile_pool(name="rv", bufs=9) as rvp,
            tc.tile_pool(name="projT", bufs=3) as projp,
            tc.tile_pool(name="small", bufs=4) as smallp,
            tc.tile_pool(name="prod", bufs=3) as prodp,
            tc.tile_pool(name="attnout", bufs=4) as attnp,
            tc.tile_pool(name="accs", bufs=1) as accs,
            tc.tile_pool(name="ps_big", bufs=4, space="PSUM") as ps_big,
            tc.tile_pool(name="ps_sc", bufs=2, space="PSUM") as ps_sc,
            tc.tile_pool(name="ps_ebc", bufs=1, space="PSUM") as ps_ebc,
            tc.tile_pool(name="ps_warm", bufs=1, space="PSUM") as ps_warm,
        ):

            rep_raw = accs.tile([R2, NPAIR], f32)
            rz_allf = accs.tile([2 * K, NPAIR], f32)

            # attn out view: row r=(bh,k) on partitions -> [r, pair, l]
            attn_view = attn_d[:].rearrange("(pr two) k l -> (two k) pr l", two=2)

            # --- constants ---
            wproj_sb = consts.tile([120, 5, R2], bf16)
            nc.sync.dma_start(out=wproj_sb, in_=wproj_d[:])
            wsc_sb = consts.tile([R2, 3, 2 * K], bf16)
            nc.sync.dma_start(out=wsc_sb, in_=wsc_d[:])
            b01_sb = consts.tile([2 * K, R2], bf16)
            nc.sync.dma_start(out=b01_sb, in_=b01_d[:])
            b01f_d_view = b01_d  # reuse bf16 dram, cast during SWDGE load
            b01f = consts.tile([2 * K, R2], f32)
            nc.gpsimd.dma_start(out=b01f, in_=b01f_d_view[:])

            blocks = []  # (pair0, W, rv tile)
            rv0a = rvp.tile([120, 1, 5, L], bf16, tag="rv")
            nc.sync.dma_start(out=rv0a, in_=reviewt[0, :, 0:1])
            blocks.append((0, 1, rv0a))
            rv0b = rvp.tile([120, 1, 5, L], bf16, tag="rv")
            nc.sync.dma_start(out=rv0b, in_=reviewt[0, :, 1:2])
            blocks.append((1, 1, rv0b))
            for g in range(1, NQUAD):
                rv = rvp.tile([120, 2, 5, L], bf16, tag="rv")
                nc.sync.dma_start(out=rv, in_=reviewt[g])
                blocks.append((2 * g, 2, rv))

            attn_chunk = None
            for pair0, W, rv in blocks:
                pc = pair0 % PAIRS_PER_CHUNK
                if pc == 0:
                    attn_chunk = attnp.tile(
                        [2 * K, PAIRS_PER_CHUNK, L], f32, tag="attn"
                    )

                # 1) projection: combined (bh, e) contraction rows tiled
                #    as 5 chunks of 120; weights loaded once per chunk;
                #    per-pair PSUM tiles for finer pipelining
                pproj_t = [ps_big.tile([R2, 512], f32, tag="pproj",
                                       name=f"pproj{p2}")
                           for p2 in range(W)]
                for c in range(5):
                    for p2 in range(W):
                        nc.tensor.matmul(
                            pproj_t[p2][:, 0:L],
                            wproj_sb[:, c, :],
                            rv[:, p2, c, :],
                            start=(c == 0),
                            stop=(c == 4),
                        )
                projT = projp.tile([R2, W, L], bf16, tag="projT")
                for p2 in range(W):
                    nc.scalar.copy(out=projT[:, p2, :], in_=pproj_t[p2][:, 0:L])

                # 2) scores: window conv as 3 shifted accumulating matmuls
                psc_t = [ps_sc.tile([2 * K, 512], f32, tag="psc",
                                    name=f"psc{p2}")
                         for p2 in range(W)]
                for p2 in range(W):
                    nc.tensor.matmul(
                        psc_t[p2][:, 0:L], wsc_sb[:, 1, :], projT[:, p2, 0:L],
                        start=True, stop=False)
                for p2 in range(W):
                    nc.tensor.matmul(
                        psc_t[p2][:, 1:L], wsc_sb[:, 0, :], projT[:, p2, 0:L - 1],
                        start=False, stop=False, skip_group_check=True)
                for p2 in range(W):
                    nc.tensor.matmul(
                        psc_t[p2][:, 0:L - 1], wsc_sb[:, 2, :], projT[:, p2, 1:L],
                        start=False, stop=True)

                # 3) softmax along l (scores are O(1e-2): no max needed);
                #    exp's accum_out is the denominator
                e_sb = smallp.tile([2 * K, W, L], bf16, tag="e")
                for p2 in range(W):
                    nc.scalar.activation(
                        out=e_sb[:, p2, :], in_=psc_t[p2][:, 0:L],
                        func=mybir.ActivationFunctionType.Exp,
                        accum_out=rz_allf[:, pair0 + p2:pair0 + p2 + 1],
                    )
                rzq = rz_allf[:, pair0:pair0 + W]
                nc.vector.reciprocal(out=rzq, in_=rzq)
                scale_eng = (nc.gpsimd if (pair0 // 2) % 2 == 0 or pair0 + W == NPAIR
                             else nc.vector)
                scale_eng.tensor_tensor(
                    out=attn_chunk[:, pc:pc + W, :], in0=e_sb,
                    in1=rzq.to_broadcast([2 * K, W, L]),
                    op=mybir.AluOpType.mult,
                )

                # 4) rep: broadcast e rows (bh,k) -> (bh,k,h), mul, reduce
                prod = prodp.tile([R2, W, L], f32, tag="prod")
                for p2 in range(W):
                    pebc = ps_ebc.tile([R2, 512], f32, tag="pebc")
                    nc.tensor.matmul(pebc[:, 0:L], b01_sb, e_sb[:, p2, :],
                                     start=True, stop=True)
                    nc.vector.tensor_tensor(
                        out=prod[:, p2, :], in0=projT[:, p2, :],
                        in1=pebc[:, 0:L], op=mybir.AluOpType.mult,
                    )
                nc.vector.tensor_reduce(
                    out=rep_raw[:, pair0:pair0 + W], in_=prod,
                    axis=mybir.AxisListType.X, op=mybir.AluOpType.add,
                )

                if pc + W == PAIRS_PER_CHUNK:
                    qq = pair0 // PAIRS_PER_CHUNK
                    nc.sync.dma_start(
                        out=attn_view[:, qq * PAIRS_PER_CHUNK:(qq + 1) * PAIRS_PER_CHUNK],
                        in_=attn_chunk,
                    )

                # rep normalization rz10[r, pair] = 1/Z[bh(r), k(r), pair]:
                # pairs 0-13 overlapped with compute, last two in the tail
                # (f32 matmul: no bf16 cast hop; N is tiny so rate is moot)
                for h0, h1 in (((0, NPAIR - 2),) if pair0 + W == NPAIR - 2
                               else ((NPAIR - 2, NPAIR),) if pair0 + W == NPAIR
                               else ()):
                    prz = ps_warm.tile([R2, h1 - h0], f32, tag="warm")
                    nc.tensor.matmul(prz, b01f, rz_allf[:, h0:h1],
                                     start=True, stop=True)
                    rep_out = smallp.tile([R2, h1 - h0], f32, tag="rpo")
                    nc.vector.tensor_tensor(out=rep_out, in0=rep_raw[:, h0:h1],
                                            in1=prz, op=mybir.AluOpType.mult)
                    rep_view = rep_d[:].rearrange(
                        "(pr two) k h -> (two k h) pr", two=2)
                    nc.sync.dma_start(out=rep_view[:, h0:h1], in_=rep_out)


            # (rep normalization emitted inside the block loop)

    nc.compile()
    return nc


def _host_prep(review, aspProj, aspEmbed):
    """Build per-core input maps (host-side shard + layout prep)."""
    review = np.asarray(review, dtype=np.float32)
    aspProj = np.asarray(aspProj, dtype=np.float32)
    aspEmbed = np.asarray(aspEmbed, dtype=np.float32)
    bf = ml_dtypes.bfloat16

    # (B, E, L) layout so E lands on SBUF partitions; bf16 halves HBM
    # bytes. The two batches of each pair are stacked into one 600-row
    # (bh, e) contraction space, tiled as 5 chunks of 120, then pre-tiled
    # per core as [group, p, pair, chunk, l] so each partition row of a
    # group load is one contiguous run (fast HWDGE descriptor gen).
    reviewt = review.transpose(0, 2, 1).astype(bf)       # (B, E, L)
    reviewt = reviewt.reshape(B // 2, 2 * E, L)          # pair-stacked rows
    reviewt = reviewt.reshape(B // 2, 5, 120, L)         # 5 chunks of 120
    reviewt = np.ascontiguousarray(
        reviewt.reshape(B // 4, 2, 5, 120, L).transpose(0, 3, 1, 2, 4))

    wflat = aspProj.transpose(1, 0, 2).reshape(E, KH)  # [e, 10k+h]
    # wproj[p, c, 50bh + f] = wflat[e, f] for row g = 120c + p = 300bh + e
    wproj = np.zeros((120, 5, R2), np.float32)
    for c in range(5):
        for p in range(120):
            g = 120 * c + p
            bh, e = divmod(g, E)
            wproj[p, c, KH * bh:KH * (bh + 1)] = wflat[e]

    # wsc[r=50bh+10k+h, o, 5bh+k] = aspEmbed[k, 3h+o]
    wsc = np.zeros((R2, 3, 2 * K), np.float32)
    for bh in range(2):
        for k in range(K):
            for h in range(H):
                wsc[KH * bh + 10 * k + h, :, K * bh + k] = aspEmbed[k, 3 * h:3 * h + 3]

    # b01[5bh+k, 50bh+10k'+h] = (k == k')
    b01 = np.zeros((2 * K, R2), np.float32)
    for bh in range(2):
        for k in range(K):
            b01[K * bh + k, KH * bh + 10 * k:KH * bh + 10 * k + H] = 1.0

    wproj, wsc, b01 = (x.astype(bf) for x in (wproj, wsc, b01))

    in_maps = []
    for core in range(N_CORES):
        in_maps.append({
            "reviewt": np.ascontiguousarray(
                reviewt[core * NQUAD:(core + 1) * NQUAD]),
            "wproj": wproj,
            "wsc": wsc,
            "b01": b01,
        })
    return in_maps


def kernel(review, aspProj, aspEmbed, _trace=False):
    global _CACHED
    from concourse.bass_utils import run_bass_kernel_spmd

    if _CACHED is None:
        _CACHED = _build()
    nc = _CACHED

    in_maps = _host_prep(review, aspProj, aspEmbed)
    res = None
    for attempt in range(3):
        try:
            res = run_bass_kernel_spmd(
                nc, in_maps, core_ids=list(range(N_CORES)), trace=_trace
            )
            break
        except Exception:
            if attempt == 2:
                raise
            import time
            time.sleep(30)

    attn = np.concatenate([res.results[c]["attn"] for c in range(N_CORES)], axis=0)
    rep = np.concatenate([res.results[c]["rep"] for c in range(N_CORES)], axis=0)
    if _trace:
        kernel.last_exec_time_ns = res.exec_time_ns
        kernel.last_results = res
    return attn, rep
